# revision 5
# baseline (speedup 1.0000x reference)
"""MoE (top-2 of 8 experts, SwiGLU FFN, D=1024 H=64) Trainium2 kernel.

Problem: nn_CellGen_730144440866.
Strategy: data-parallel over tokens — each of the 8 NeuronCores processes
1024 of the 8192 tokens independently (routing is per-token; no collectives).

Per-core layout is feature-major ("transposed"): activations live as
xT [D, tok] so every matmul contracts over the partition axis with zero
on-device transposes in the hot path. Expert pairs are packed so the
128x128 PE array is always full despite H=64:
  mm1: psumA[128, tok] = [h1_{2p}; h1_{2p+1}]  (lhsT = [W1_2p | W1_2p+1])
       psumB[128, tok] = [h3_{2p}; h3_{2p+1}]
  g    = silu(psumA) * psumB * w_broadcast     (w via tiny selector matmul)
  mm2: out[tok, D] += g_p.T @ [W2_2p; W2_2p+1] (K=128 = two experts' H)
The cumsum/classifier head only needs the per-expert contribution at token 0
of each batch row: contrib[e, t0] . Wc = g_e[:, t0] . (W2[e] @ Wc), so the
host precomputes v[e] = W2[e] @ Wc and a tiny [K=128, N=2] matmul per pair
yields the per-expert dots; the host does the 8-element cumsum.

Numerics: FFN matmuls run in float32r (TF32-like, ~1.5e-4 rel err, 4x the
fp32 rate); the gate matmul runs in exact fp32 because the dataset has
routing-weight gaps down to 5e-7 and a top-2 selection flip would be a
full-magnitude error at that token.
"""

import sys

sys.path.insert(0, "/opt/trn_rl_repo")

import numpy as np

import concourse.bass as bass
import concourse.tile as tile
from concourse import bacc, mybir
from concourse.bass_utils import run_bass_kernel_spmd

F32 = mybir.dt.float32
F32R = mybir.dt.float32r
AF = mybir.ActivationFunctionType
ALU = mybir.AluOpType
AX = mybir.AxisListType

N_CORES = 8
B, S, D, E, H = 4, 2048, 1024, 8, 64
T = B * S                 # 8192 tokens total
NT = T // N_CORES         # 1024 tokens per core
NP = E // 2               # 4 expert pairs
KC = D // 128             # 8 contraction chunks
CH = 512                  # token chunk (matmul moving-dim / PSUM bank)
NCH = NT // CH            # 2 token chunks per core

_CACHE = {}


def _build():
    nc = bacc.Bacc()

    XT = nc.dram_tensor("XT", [D, NT], F32, kind="ExternalInput")
    WGT = nc.dram_tensor("WGT", [D, E], F32, kind="ExternalInput")
    W11 = nc.dram_tensor("W11", [NP, 128, D], F32R, kind="ExternalInput")
    W33 = nc.dram_tensor("W33", [NP, 128, D], F32R, kind="ExternalInput")
    W2S = nc.dram_tensor("W2S", [NP, 128, D], F32R, kind="ExternalInput")
    VP8 = nc.dram_tensor("VP8", [128, E], F32R, kind="ExternalInput")
    SEL8 = nc.dram_tensor("SEL8", [NP, E, 128], F32R, kind="ExternalInput")
    IDT = nc.dram_tensor("IDT", [128, 128], F32, kind="ExternalInput")

    OUT = nc.dram_tensor("OUT", [NT, D], F32, kind="ExternalOutput")
    GATE = nc.dram_tensor("GATE", [NT, E], F32, kind="ExternalOutput")
    RW = nc.dram_tensor("RW", [NT, E], F32, kind="ExternalOutput")
    D8 = nc.dram_tensor("D8", [1, E], F32, kind="ExternalOutput")

    with tile.TileContext(nc) as tc:
        with (
            tc.tile_pool(name="const", bufs=1) as cp,
            tc.tile_pool(name="gsm", bufs=2) as gp,
            tc.tile_pool(name="work", bufs=2) as wp,
            tc.tile_pool(name="outp", bufs=4) as op_,
        ):
            # ---- resident loads ----
            xt32 = []
            xtr = []
            for k in range(KC):
                t32 = cp.tile([128, NT], F32, tag=f"xt32_{k}")
                nc.sync.dma_start(t32[:], XT[k * 128:(k + 1) * 128, :])
                xt32.append(t32)
                tr = cp.tile([128, NT], F32R, tag=f"xtr_{k}")
                nc.vector.tensor_copy(tr[:], t32[:])
                xtr.append(tr)
            wgt = []
            for k in range(KC):
                t = cp.tile([128, E], F32, tag=f"wgt_{k}")
                nc.sync.dma_start(t[:], WGT[k * 128:(k + 1) * 128, :])
                wgt.append(t)
            w11, w33, w2s = [], [], []
            for p in range(NP):
                t1 = cp.tile([128, D], F32R, tag=f"w11_{p}")
                nc.sync.dma_start(t1[:], W11[p])
                w11.append(t1)
                t3 = cp.tile([128, D], F32R, tag=f"w33_{p}")
                nc.sync.dma_start(t3[:], W33[p])
                w33.append(t3)
                t2 = cp.tile([128, D], F32R, tag=f"w2s_{p}")
                nc.sync.dma_start(t2[:], W2S[p])
                w2s.append(t2)
            vp8 = cp.tile([128, E], F32R, tag="vp8")
            nc.sync.dma_start(vp8[:], VP8[:])
            sel8 = []
            for p in range(NP):
                t = cp.tile([E, 128], F32R, tag=f"sel8_{p}")
                nc.sync.dma_start(t[:], SEL8[p])
                sel8.append(t)
            idt = cp.tile([128, 128], F32, tag="idt")
            nc.sync.dma_start(idt[:], IDT[:])
            wt = cp.tile([E, NT], F32R, tag="wt")

            # ---- gating: softmax + top-2 renorm, per 128-token block ----
            with tc.tile_pool(name="pg", bufs=2, space="PSUM") as pg:
                for m in range(NT // 128):
                    psg = pg.tile([128, E], F32, tag="psg")
                    for k in range(KC):
                        nc.tensor.matmul(
                            psg[:], xt32[k][:, m * 128:(m + 1) * 128], wgt[k][:],
                            start=(k == 0), stop=(k == KC - 1),
                        )
                    gsb = gp.tile([128, E], F32, tag="gsb")
                    nc.scalar.copy(gsb[:], psg[:])
                    nc.sync.dma_start(GATE[m * 128:(m + 1) * 128, :], gsb[:])

                    negmax = gp.tile([128, 1], F32, tag="negmax")
                    nc.vector.reduce_max(negmax[:], psg[:], axis=AX.X, negate=True)
                    esb = gp.tile([128, E], F32, tag="esb")
                    nc.scalar.activation(esb[:], psg[:], AF.Exp, bias=negmax[:])
                    ssum = gp.tile([128, 1], F32, tag="ssum")
                    nc.vector.reduce_sum(ssum[:], esb[:], axis=AX.X)
                    sinv = gp.tile([128, 1], F32, tag="sinv")
                    nc.vector.reciprocal(sinv[:], ssum[:])
                    rwsb = gp.tile([128, E], F32, tag="rwsb")
                    nc.vector.tensor_scalar_mul(rwsb[:], esb[:], sinv[:])
                    nc.sync.dma_start(RW[m * 128:(m + 1) * 128, :], rwsb[:])

                    m1 = gp.tile([128, 1], F32, tag="m1")
                    nc.vector.reduce_max(m1[:], rwsb[:], axis=AX.X)
                    eqm = gp.tile([128, E], F32, tag="eqm")
                    nc.vector.tensor_scalar(
                        eqm[:], rwsb[:], m1[:], -2.0, op0=ALU.is_equal, op1=ALU.mult
                    )
                    nmx = gp.tile([128, E], F32, tag="nmx")
                    nc.vector.tensor_add(nmx[:], rwsb[:], eqm[:])
                    m2 = gp.tile([128, 1], F32, tag="m2")
                    nc.vector.reduce_max(m2[:], nmx[:], axis=AX.X)
                    s12 = gp.tile([128, 1], F32, tag="s12")
                    nc.vector.tensor_add(s12[:], m1[:], m2[:])
                    inv12 = gp.tile([128, 1], F32, tag="inv12")
                    nc.vector.reciprocal(inv12[:], s12[:])
                    selm = gp.tile([128, E], F32, tag="selm")
                    nc.vector.tensor_scalar(
                        selm[:], rwsb[:], m2[:], None, op0=ALU.is_ge
                    )
                    wd = gp.tile([128, E], F32, tag="wd")
                    nc.vector.scalar_tensor_tensor(
                        wd[:], rwsb[:], inv12[:], selm[:], op0=ALU.mult, op1=ALU.mult
                    )
                    pst = pg.tile([E, 128], F32, tag="pst")
                    nc.tensor.transpose(pst[:], wd[:], idt[:])
                    nc.vector.tensor_copy(wt[:, m * 128:(m + 1) * 128], pst[:])

            # ---- expert FFN main loop ----
            with (
                tc.tile_pool(name="pa", bufs=2, space="PSUM") as pa,
                tc.tile_pool(name="pb", bufs=2, space="PSUM") as pb,
                tc.tile_pool(name="pw", bufs=1, space="PSUM") as pw,
                tc.tile_pool(name="po", bufs=2, space="PSUM") as po,
                tc.tile_pool(name="pd", bufs=1, space="PSUM") as pd,
            ):
                psd = pd.tile([128, E], F32, tag="psd")
                gtiles = [None] * NP
                for c in range(NCH):
                    tok = slice(c * CH, (c + 1) * CH)
                    for p in range(NP):
                        psA = pa.tile([128, CH], F32, tag="psA")
                        for k in range(KC):
                            nc.tensor.matmul(
                                psA[:], w11[p][:, k * 128:(k + 1) * 128],
                                xtr[k][:, tok],
                                start=(k == 0), stop=(k == KC - 1),
                            )
                        psB = pb.tile([128, CH], F32, tag="psB")
                        for k in range(KC):
                            nc.tensor.matmul(
                                psB[:], w33[p][:, k * 128:(k + 1) * 128],
                                xtr[k][:, tok],
                                start=(k == 0), stop=(k == KC - 1),
                            )
                        sA = wp.tile([128, CH], F32, tag="sA")
                        nc.scalar.activation(sA[:], psA[:], AF.Silu)
                        psW = pw.tile([128, CH], F32, tag="psW")
                        nc.tensor.matmul(
                            psW[:], sel8[p][:], wt[:, tok],
                            start=True, stop=True,
                        )
                        graw = wp.tile([128, CH], F32, tag="graw")
                        nc.vector.tensor_tensor(graw[:], sA[:], psB[:], op=ALU.mult)
                        g = wp.tile([128, CH], F32R, tag=f"g{p}")
                        nc.vector.tensor_tensor(g[:], graw[:], psW[:], op=ALU.mult)
                        gtiles[p] = g
                        if c == 0:
                            nc.tensor.matmul(
                                psd[:, 2 * p:2 * p + 2], g[:, 0:128],
                                vp8[:, 2 * p:2 * p + 2], start=True, stop=True,
                            )
                    for m in range(CH // 128):
                        row = slice(c * CH + m * 128, c * CH + (m + 1) * 128)
                        for dch in range(2):
                            psO = po.tile([128, 512], F32, tag="psO")
                            for p in range(NP):
                                nc.tensor.matmul(
                                    psO[:], gtiles[p][:, m * 128:(m + 1) * 128],
                                    w2s[p][:, dch * 512:(dch + 1) * 512],
                                    start=(p == 0), stop=(p == NP - 1),
                                )
                            ob = op_.tile([128, 512], F32, tag="ob")
                            nc.scalar.copy(ob[:], psO[:])
                            nc.sync.dma_start(
                                OUT[row, dch * 512:(dch + 1) * 512], ob[:]
                            )

                dsb = gp.tile([1, E], F32, tag="dsb")
                nc.scalar.copy(dsb[:], psd[0:1, :])
                nc.sync.dma_start(D8[:], dsb[:])

    nc.compile()
    return nc


def _prep_shared(Wg, W1, W2, W3, Wc):
    Wg = np.asarray(Wg, np.float32)
    W1 = np.asarray(W1, np.float32)
    W2 = np.asarray(W2, np.float32)
    W3 = np.asarray(W3, np.float32)
    Wc = np.asarray(Wc, np.float32)

    WgT = np.ascontiguousarray(Wg.T)  # [D, E]

    def pack_pair_dK(W):  # W [E, D, H] -> [NP, 128, D] host layout [p][dp][k*128+m]
        out = np.empty((NP, 128, D), np.float32)
        for p in range(NP):
            cat = np.concatenate([W[2 * p], W[2 * p + 1]], axis=1)  # [D, 128]
            out[p] = cat.reshape(KC, 128, 128).transpose(1, 0, 2).reshape(128, D)
        return out

    W11 = pack_pair_dK(W1)
    W33 = pack_pair_dK(W3)
    W2S = np.empty((NP, 128, D), np.float32)
    for p in range(NP):
        W2S[p] = np.concatenate([W2[2 * p], W2[2 * p + 1]], axis=0)  # [128, D]

    v = np.einsum("ehd,d->eh", W2, Wc).astype(np.float32)  # [E, H]
    VP8 = np.zeros((128, E), np.float32)
    for p in range(NP):
        VP8[0:64, 2 * p] = v[2 * p]
        VP8[64:128, 2 * p + 1] = v[2 * p + 1]

    SEL8 = np.zeros((NP, E, 128), np.float32)
    for p in range(NP):
        SEL8[p, 2 * p, 0:64] = 1.0
        SEL8[p, 2 * p + 1, 64:128] = 1.0
    IDT = np.eye(128, dtype=np.float32)
    return dict(WGT=WgT, W11=W11, W33=W33, W2S=W2S, VP8=VP8, SEL8=SEL8, IDT=IDT)


def build_in_maps(x, Wg, W1, W2, W3, Wc):
    shared = _prep_shared(Wg, W1, W2, W3, Wc)
    xf = np.asarray(x, np.float32).reshape(T, D)
    in_maps = []
    for i in range(N_CORES):
        xT = np.ascontiguousarray(xf[i * NT:(i + 1) * NT].T)  # [D, NT]
        in_maps.append({"XT": xT, **shared})
    return in_maps


def assemble(results, bc):
    final = np.concatenate([r["OUT"] for r in results], axis=0).reshape(B, S, D)
    gate = np.concatenate([r["GATE"] for r in results], axis=0)  # [T, E]
    rw = np.concatenate([r["RW"] for r in results], axis=0).reshape(B, S, E)
    bc = np.asarray(bc, np.float32)
    expert_logits = np.empty((E, B, 1), np.float32)
    for b in range(B):
        d = results[2 * b]["D8"][0]  # [E] per-expert contrib(token0_b) . Wc
        expert_logits[:, b, 0] = np.cumsum(d) + bc[0]
    return final, gate, expert_logits, rw


def kernel(x, tgt_pad, Wg, W1, W2, W3, Wc, bc):
    if "nc" not in _CACHE:
        _CACHE["nc"] = _build()
    nc = _CACHE["nc"]
    in_maps = build_in_maps(x, Wg, W1, W2, W3, Wc)
    res = run_bass_kernel_spmd(nc, in_maps, core_ids=list(range(N_CORES)))
    return assemble(res.results, bc)
